# revision 5
# baseline (speedup 1.0000x reference)
"""Trainium2 Bass kernel for nn_EnhancedCausalModel.

Computation (see reference): an MLP (288->128->64->32) evaluated for 18
action-variants per (agent, batch, time) token, followed by a KL divergence
between softmax(p_with) and softmax(p_without), averaged over the action dim.

Sharding: agents (N=16) split across 8 cores, 2 agents/core -> 2048 base
tokens/core, each with 18 variants.

Device-side algebra per core (features on partitions, tokens on free dim,
processed in 4 base tiles of 512 tokens):
  h_obs = W1[:256].T @ obs.T                       (shared across variants)
  per variant v:  pre1_v = W1[256:].T @ act_v  (+)  I @ h_obs   (PSUM accum)
                  h1_v = relu(pre1_v + b1)         (single PSUM-exit op)
                  h2_v = relu(W2.T @ h1_v + b2)    (2 variants col-packed)
                  p_v  = W3.T @ h2_v               (row-packed pairs; the 17
                         "without" variants accumulate directly into PSUM)
  KL stats per token (32 actions on a partition block):
      Ez = sum_a exp(z_a+b3), S1 = sum_a exp(z_a+b3)*(z_a-w_a),
      Ew = sum_a exp(w_a+b3)   where z = p_without, w = p_with
  reduced over the 32-partition blocks with a block-diagonal ones matmul.
Host finishes:  influence = (S1/Ez - log Ez + log Ew) / 32  (in float64).
"""

import numpy as np

import concourse.bass as bass
import concourse.mybir as mybir
import concourse.tile as tile
from concourse import bacc
from concourse.bass_utils import run_bass_kernel_spmd

F32 = mybir.dt.float32
AF = mybir.ActivationFunctionType
ALU = mybir.AluOpType

B, T = 16, 64
N_AG, D_OBS, D_ACT = 16, 256, 32
S_CF = 16
H1, H2 = 128, 64
N_CORES = 8
AG_PER = N_AG // N_CORES          # 2 agents per core
TOK = AG_PER * B * T              # 2048 tokens per core
TILE = 512
NT = TOK // TILE                  # 4 base tiles
NS = 9                            # action-variant pair slots per base tile
NPB = (NS + 1) // 2               # pair-blocks in the input block (5)
INW = 2 * TILE + NPB * TILE       # input block cols per base tile (3584)
INV_S1 = 1.0 / (S_CF + 1)         # 1/17

# const block column offsets
C_W1A0, C_W1A1, C_W1B4, C_EYE, C_W2D, C_W3R, C_ONES, C_BV = (
    0, 128, 256, 384, 512, 640, 672, 676)
C_TOT = 679


def build_nc():
    nc = bacc.Bacc("TRN2", target_bir_lowering=False, debug=False,
                   num_devices=N_CORES)

    cblk = nc.dram_tensor("cblk", [128, C_TOT], F32, kind="ExternalInput").ap()
    inblk = nc.dram_tensor("inblk", [128, NT * INW], F32,
                           kind="ExternalInput").ap()
    stats = nc.dram_tensor("stats", [4, 3, TILE], F32,
                           kind="ExternalOutput").ap()

    with tile.TileContext(nc) as tc:
        with (
            tc.tile_pool(name="const", bufs=1) as cpool,
            tc.tile_pool(name="inp", bufs=2) as ipool,
            tc.tile_pool(name="acts", bufs=3) as apool,
            tc.tile_pool(name="fin", bufs=1) as fpool,
            tc.tile_pool(name="pp", bufs=1, space="PSUM") as pp,
        ):
            cb = cpool.tile([128, C_TOT], F32)
            nc.gpsimd.dma_start(out=cb[:], in_=cblk[:])
            w1a0 = cb[:, C_W1A0:C_W1A0 + 128]
            w1a1 = cb[:, C_W1A1:C_W1A1 + 128]
            w1b4 = cb[:, C_W1B4:C_W1B4 + 128]
            eye = cb[:, C_EYE:C_EYE + 128]
            w2d = cb[:, C_W2D:C_W2D + 128]
            w3r = cb[:, C_W3R:C_W3R + 32]
            onesb = cb[:, C_ONES:C_ONES + 4]
            b1 = cb[:, C_BV:C_BV + 1]
            b2d = cb[:, C_BV + 1:C_BV + 2]
            b3r = cb[:, C_BV + 2:C_BV + 3]

            pw = pp.tile([128, TILE], F32, tag="pw")
            woA = pp.tile([128, TILE], F32, tag="woA")
            woB = pp.tile([128, TILE], F32, tag="woB")
            mm = nc.tensor.matmul

            with (
                tc.tile_pool(name="pv", bufs=2, space="PSUM") as vp,
                tc.tile_pool(name="pq", bufs=1, space="PSUM") as qp,
            ):
                for i in range(NT):
                    inb = ipool.tile([128, INW], F32, tag="inb")
                    nc.sync.dma_start(
                        out=inb[:], in_=inblk[:, i * INW:(i + 1) * INW])
                    obs0 = inb[:, 0:TILE]
                    obs1 = inb[:, TILE:2 * TILE]

                    # h_obs = W1a.T @ obs (K=256 in two chunks)
                    ps_h = qp.tile([128, TILE], F32, tag="q")
                    mm(ps_h[:], w1a0, obs0, start=True, stop=False)
                    mm(ps_h[:], w1a1, obs1, start=False, stop=True)
                    h_obs = apool.tile([128, TILE], F32, tag="h_obs")
                    nc.scalar.activation(h_obs[:], ps_h[:], AF.Copy)

                    # zero-action variant: h1 = relu(h_obs + b1)
                    h1z = apool.tile([128, TILE], F32, tag="h1z")
                    nc.scalar.activation(h1z[:], h_obs[:], AF.Relu, bias=b1)
                    ps_qz = qp.tile([128, TILE], F32, tag="q")
                    mm(ps_qz[0:64, :], w2d[:, 0:64], h1z[:],
                       tile_position=(0, 0))
                    h2z = apool.tile([64, TILE], F32, tag="h2z")
                    nc.scalar.activation(h2z[:], ps_qz[0:64, :], AF.Relu,
                                         bias=b2d[0:64])
                    mm(woA[32 * i:32 * i + 32, :], w3r[0:64, :], h2z[:],
                       tile_position=(0, 32 * i), start=True, stop=False,
                       skip_group_check=True)

                    for k in range(NS):
                        pair = k < 8
                        p_blk = k // 2
                        ra = 64 * (k % 2)
                        acols = slice(2 * TILE + p_blk * TILE,
                                      2 * TILE + (p_blk + 1) * TILE)
                        pv = vp.tile([128, 2, TILE], F32, tag="pv")
                        # L1 action part (K=32, row-tiled) + obs via identity
                        mm(pv[:, 0, :], w1b4[ra:ra + 32, :],
                           inb[ra:ra + 32, acols],
                           tile_position=(ra, 0), start=True, stop=False)
                        if pair:
                            mm(pv[:, 1, :], w1b4[ra + 32:ra + 64, :],
                               inb[ra + 32:ra + 64, acols],
                               tile_position=(ra + 32, 0),
                               start=True, stop=False)
                        mm(pv[:, 0, :], eye, h_obs[:], start=False, stop=True)
                        if pair:
                            mm(pv[:, 1, :], eye, h_obs[:],
                               start=False, stop=True)
                        # h1 = relu(pre + b1), one PSUM-exit op
                        h1 = apool.tile([128, 2, TILE], F32, tag="h1")
                        nv = 2 if pair else 1
                        if k % 2 == 0:
                            nc.scalar.activation(h1[:, 0:nv, :],
                                                 pv[:, 0:nv, :], AF.Relu,
                                                 bias=b1)
                        else:
                            nc.vector.tensor_scalar(
                                h1[:, 0:nv, :], pv[:, 0:nv, :], b1, 0.0,
                                op0=ALU.add, op1=ALU.max)
                        # L2 (col-packed pair)
                        ps_q = qp.tile([128, TILE], F32, tag="q")
                        mm(ps_q[0:64, :], w2d[:, 0:64], h1[:, 0, :],
                           tile_position=(0, 0))
                        if pair:
                            mm(ps_q[64:128, :], w2d[:, 64:128], h1[:, 1, :],
                               tile_position=(0, 64))
                        h2 = apool.tile([128, TILE], F32, tag="h2")
                        np_ = 128 if pair else 64
                        if k % 2 == 1:
                            nc.scalar.activation(h2[0:np_, :], ps_q[0:np_, :],
                                                 AF.Relu, bias=b2d[0:np_])
                        else:
                            nc.vector.tensor_scalar(
                                h2[0:np_, :], ps_q[0:np_, :], b2d[0:np_], 0.0,
                                op0=ALU.add, op1=ALU.max)
                        # L3 (row-packed pair); "without" variants accumulate
                        if k == 0:
                            mm(pw[32 * i:32 * i + 32, :], w3r[0:64, :],
                               h2[0:64, :], tile_position=(0, 32 * i),
                               start=True, stop=True, skip_group_check=True)
                        else:
                            mm(woA[32 * i:32 * i + 32, :], w3r[0:64, :],
                               h2[0:64, :], tile_position=(0, 32 * i),
                               start=False, stop=(k == 8),
                               skip_group_check=True)
                        if pair:
                            mm(woB[32 * i:32 * i + 32, :], w3r[64:128, :],
                               h2[64:128, :], tile_position=(64, 32 * i),
                               start=(k == 0), stop=(k == 7),
                               skip_group_check=True)

            # ---- KL stats over all 4 base tiles at once ([4x32, 512]) ----
            with tc.tile_pool(name="pr", bufs=1, space="PSUM") as rp:
                woBs = fpool.tile([128, TILE], F32)
                nc.vector.tensor_scalar_mul(woBs[:], woB[:], INV_S1)
                zz = fpool.tile([128, TILE], F32)
                nc.vector.scalar_tensor_tensor(
                    zz[:], woA[:], INV_S1, woBs[:],
                    op0=ALU.mult, op1=ALU.add)
                stag = fpool.tile([128, 3, TILE], F32)
                nc.scalar.activation(stag[:, 0, :], zz[:], AF.Exp, bias=b3r)
                dd = fpool.tile([128, TILE], F32)
                nc.vector.scalar_tensor_tensor(
                    dd[:], pw[:], -1.0, zz[:], op0=ALU.mult, op1=ALU.add)
                nc.vector.tensor_mul(stag[:, 1, :], stag[:, 0, :], dd[:])
                nc.scalar.activation(stag[:, 2, :], pw[:], AF.Exp, bias=b3r)
                ps_red = rp.tile([4, 3, TILE], F32)
                for q in range(3):
                    mm(ps_red[:, q, :], onesb, stag[:, q, :])
                out_sb = fpool.tile([4, 3, TILE], F32)
                for q in range(3):
                    nc.vector.tensor_copy(out_sb[:, q, :], ps_red[:, q, :])
                nc.sync.dma_start(out=stats[:], in_=out_sb[:])

    nc.compile()
    return nc


def prep_shared(W1, b1, W2, b2, W3, b3):
    cblk = np.zeros((128, C_TOT), np.float32)
    cblk[:, C_W1A0:C_W1A0 + 128] = W1[0:128]
    cblk[:, C_W1A1:C_W1A1 + 128] = W1[128:256]
    w1b = W1[D_OBS:]
    cblk[:, C_W1B4:C_W1B4 + 128] = np.vstack([w1b] * 4)
    cblk[:, C_EYE:C_EYE + 128] = np.eye(128, dtype=np.float32)
    cblk[:, C_W2D:C_W2D + 128] = np.concatenate([W2, W2], axis=1)
    cblk[:, C_W3R:C_W3R + 32] = np.vstack([W3, W3])
    for i in range(4):
        cblk[32 * i:32 * i + 32, C_ONES + i] = 1.0
    cblk[:, C_BV] = b1
    cblk[:, C_BV + 1] = np.concatenate([b2, b2])
    cblk[:, C_BV + 2] = np.tile(b3, 4)
    return dict(cblk=cblk)


def prep_core(obs, actions, cf_actions, c):
    n0 = AG_PER * c
    obs_c = obs[:, :, n0:n0 + AG_PER, :]                    # [B,T,2,D]
    obs_t = np.transpose(obs_c, (3, 2, 0, 1)).reshape(D_OBS, TOK)
    act_w = np.transpose(actions[:, :, n0:n0 + AG_PER, :],
                         (3, 2, 0, 1)).reshape(D_ACT, TOK)
    cf_c = cf_actions[n0:n0 + AG_PER]                       # [2,S,B,T,A]
    cf_tok = np.transpose(cf_c, (4, 1, 0, 2, 3)).reshape(D_ACT, S_CF, TOK)
    # slot composition: (with, cf0), (cf1,cf2)..(cf13,cf14), (cf15, -)
    vA = [act_w] + [cf_tok[:, 2 * k - 1] for k in range(1, 8)] + [cf_tok[:, 15]]
    vB = [cf_tok[:, 0]] + [cf_tok[:, 2 * k] for k in range(1, 8)]
    inblk = np.zeros((128, NT * INW), np.float32)
    for i in range(NT):
        tokc = slice(i * TILE, (i + 1) * TILE)
        base = i * INW
        inblk[0:128, base:base + TILE] = obs_t[0:128, tokc]
        inblk[0:128, base + TILE:base + 2 * TILE] = obs_t[128:256, tokc]
        for k in range(NS):
            p_blk, ra = k // 2, 64 * (k % 2)
            blk = slice(base + 2 * TILE + p_blk * TILE,
                        base + 2 * TILE + (p_blk + 1) * TILE)
            inblk[ra:ra + 32, blk] = vA[k][:, tokc]
            if k < 8:
                inblk[ra + 32:ra + 64, blk] = vB[k][:, tokc]
    return dict(inblk=inblk)


def stats_to_influence(stats):
    s = stats.astype(np.float64)
    ez = s[:, 0, :].reshape(TOK)
    s1 = s[:, 1, :].reshape(TOK)
    ew = s[:, 2, :].reshape(TOK)
    return ((s1 / ez - np.log(ez) + np.log(ew)) / float(D_ACT)).astype(np.float32)


_NC_CACHE = {}


def run_device(inputs, trace=False):
    if "nc" not in _NC_CACHE:
        _NC_CACHE["nc"] = build_nc()
    nc = _NC_CACHE["nc"]
    shared = prep_shared(np.asarray(inputs["W1"]), np.asarray(inputs["b1"]),
                         np.asarray(inputs["W2"]), np.asarray(inputs["b2"]),
                         np.asarray(inputs["W3"]), np.asarray(inputs["b3"]))
    maps = []
    for c in range(N_CORES):
        m = dict(shared)
        m.update(prep_core(np.asarray(inputs["obs"]),
                           np.asarray(inputs["actions"]),
                           np.asarray(inputs["cf_actions"]), c))
        maps.append(m)
    res = run_bass_kernel_spmd(nc, maps, list(range(N_CORES)), trace=trace)
    return res


def kernel(**inputs):
    res = run_device(inputs, trace=False)
    out = np.empty((B, T, N_AG), np.float32)
    for c in range(N_CORES):
        infl = stats_to_influence(res.results[c]["stats"])
        r = infl.reshape(AG_PER, B, T)
        for a in range(AG_PER):
            out[:, :, AG_PER * c + a] = r[a]
    return out
